# revision 9
# baseline (speedup 1.0000x reference)
"""AlphaFold2 axial (row/column) MSA attention on 8 Trainium2 NeuronCores.

Problem: x (1, 32768, 256) = 128 MSA rows x 256 columns x dim 256.
  - width attention: softmax attention across the 128 rows, independent per
    column (256 independent length-128 sequences), 8 heads x 64.
  - height attention: "tied" attention across the 256 columns: logits are
    summed over all 128 rows, one (256x256) softmax per head shared by all
    rows.

Sharding (8 cores):
  - width: each core owns 32 columns (fully local).
  - height: each core owns 16 rows; per-core partial logits (8,256,256) are
    AllReduce'd (fp8, 0.5MB) across cores, softmax replicated, attn*V local.

Implementation (v4):
  - activations feature-major ("xT"), prepared host-side; projections and
    q.k^T need no on-device transpose.
  - ALL q/k projections run in fp8e4m3 with DoubleRow matmuls: the host
    supplies x and the q/k weights in the [128, 2, n] pair-interleaved
    layout, so one instruction contracts the full model dim (2x128) at 2x PE
    throughput.  q/k only feed softmax logits (~N(0, 0.1) after scaling), so
    fp8 error washes out in the near-uniform softmax; v and the output
    projections stay bf16 for accuracy.
  - the tied height logits additionally use DoubleRow to pack TWO MSA rows
    per instruction (contraction (row-pair, head-dim) via 3D APs on the fp8
    q^T/k^T tiles), and the AllReduce payload is fp8 (0.5MB).  hdots(f) is
    emitted behind the projections of f+1 so the in-order PE queue never
    stalls on the PSUM->SBUF cast copies; the AllReduce launches ~25us in.
  - width attention per column uses ONE [128, 1024] PSUM tile: dots -> one
    fused-scale exp -> attn*V overwrites bank 0 / ones-matmul denominators
    bank 1 -> reciprocal + normalize-multiply evacuate.  Normalization is
    deferred to the PSUM->SBUF copy; no max-subtraction.
  - the height v-projections, the width output projection, and the height
    exp(logits) are interleaved into the width column loop; the AllReduce
    result is fetched on the gpsimd DGE queue right after the collective.
  - DMAs are consolidated with 3D access patterns (one DMA per weight
    tensor, two per x variant, one per output chunk pair) because each
    DMACopy costs ~1us of in-order queue occupancy on top of the transfer.
  - outputs are written feature-major in bf16 and transposed/summed on host.

PSUM rules honored: a matmul accumulation chain fully finishes before
another chain's start=True touches the same bank (start clears has_written
bank-wide; data persists).
"""

import sys

for _p in ("/opt/trn_rl_repo",):
    if _p not in sys.path:
        sys.path.append(_p)

import numpy as np
import ml_dtypes

import concourse.bass as bass
import concourse.mybir as mybir
import concourse.tile as tile
from concourse import bacc
from concourse.bass_utils import run_bass_kernel_spmd

BF16 = mybir.dt.bfloat16
F32 = mybir.dt.float32
FP8 = mybir.dt.float8e4
NPBF16 = ml_dtypes.bfloat16
NPFP8 = ml_dtypes.float8_e4m3fn
EXP = mybir.ActivationFunctionType.Exp
DR = mybir.MatmulPerfMode.DoubleRow

N_CORES = 8
H_ROWS = 128          # MSA rows
W_COLS = 256          # sequence length (columns)
D = 256               # model dim
NH = 8                # heads
DH = 64               # head dim
INNER = NH * DH       # 512
WPC = W_COLS // N_CORES   # 32 columns per core
RPC = H_ROWS // N_CORES   # 16 rows per core
T = 4096              # tokens per shard (WPC*H_ROWS == RPC*W_COLS)
SCALE = DH ** -0.5                   # 0.125
TIE_SCALE = SCALE * (H_ROWS ** -0.5)


def _ap(h):
    return h.ap()


def build_bass(loop=1):
    nc = bacc.Bacc("TRN2", target_bir_lowering=False, debug=False,
                   num_devices=N_CORES)

    # ---- per-core I/O ----
    # bf16 activations (for the v projections) + fp8 pair-interleaved copies
    # (for the q/k projections); q/k weights only exist in fp8 form.
    xw = _ap(nc.dram_tensor("xw", [D, T], BF16, kind="ExternalInput"))
    xr = _ap(nc.dram_tensor("xr", [D, T], BF16, kind="ExternalInput"))
    xw8 = _ap(nc.dram_tensor("xw8", [128, 2 * T], FP8, kind="ExternalInput"))
    xr8 = _ap(nc.dram_tensor("xr8", [128, 2 * T], FP8, kind="ExternalInput"))
    wq8 = _ap(nc.dram_tensor("wq8", [128, 2 * INNER], FP8, kind="ExternalInput"))
    wk8 = _ap(nc.dram_tensor("wk8", [128, 2 * INNER], FP8, kind="ExternalInput"))
    hq8 = _ap(nc.dram_tensor("hq8", [128, 2 * INNER], FP8, kind="ExternalInput"))
    hk8 = _ap(nc.dram_tensor("hk8", [128, 2 * INNER], FP8, kind="ExternalInput"))
    wv = _ap(nc.dram_tensor("wv", [D, INNER], BF16, kind="ExternalInput"))
    wo = _ap(nc.dram_tensor("wo", [INNER, D], BF16, kind="ExternalInput"))
    hv = _ap(nc.dram_tensor("hv", [D, INNER], BF16, kind="ExternalInput"))
    ho = _ap(nc.dram_tensor("ho", [INNER, D], BF16, kind="ExternalInput"))
    w_out_t = _ap(nc.dram_tensor("w_out_t", [D, T], BF16, kind="ExternalOutput"))
    h_out_t = _ap(nc.dram_tensor("h_out_t", [D, T], BF16, kind="ExternalOutput"))

    with tile.TileContext(nc) as tc:
        for it in range(loop):
            # collective buffers must be distinct per unrolled iteration
            cc_in = _ap(nc.dram_tensor(f"cc_in{it}", [128, NH * 512], FP8,
                                       kind="Internal"))
            cc_out = _ap(nc.dram_tensor(f"cc_out{it}", [128, NH * 512], FP8,
                                        kind="Internal", addr_space="Shared"))
            build_tile_kernel(tc, xw, xr, xw8, xr8, wq8, wk8, hq8, hk8,
                              wv, wo, hv, ho, w_out_t, h_out_t, cc_in, cc_out)

    nc.compile()
    return nc


def build_tile_kernel(tc, xw, xr, xw8, xr8, wq8, wk8, hq8, hk8,
                      wv, wo, hv, ho, w_out_t, h_out_t, cc_in, cc_out):
    from contextlib import ExitStack

    nc = tc.nc
    ctx = ExitStack()

    # round-robin PSUM->SBUF copy engine (only ACT/DVE can read PSUM)
    _eng = [0]

    def copy_ps(out, in_):
        _eng[0] = (_eng[0] + 1) % 5
        if _eng[0] in (1, 3):
            nc.vector.tensor_copy(out=out, in_=in_)
        else:
            nc.scalar.copy(out=out, in_=in_)

    consts = ctx.enter_context(tc.tile_pool(name="consts", bufs=1))
    dotsAp = ctx.enter_context(tc.tile_pool(name="dotsAp", bufs=1))
    vhp = ctx.enter_context(tc.tile_pool(name="vhp", bufs=1))

    # ---- constants / weights into SBUF (phase-A dependencies first, few
    # large DMAs -- each DMACopy costs ~1us of in-order queue occupancy) ----
    def load_fp8_w(ap_in, name):  # [128, 2, INNER] pair-interleaved
        t = consts.tile([128, 2, INNER], FP8, name=name)
        nc.sync.dma_start(out=t, in_=ap_in)
        return t

    hq8_sb = load_fp8_w(hq8, "hq8")
    hk8_sb = load_fp8_w(hk8, "hk8")
    xr8_sb = consts.tile([128, 2, T], FP8, name="xr8")
    xr8_v = xr8.rearrange("p (two t) -> p two t", two=2)
    for half in range(2):
        nc.sync.dma_start(
            out=xr8_sb[:, :, half * 2048:(half + 1) * 2048],
            in_=xr8_v[:, :, half * 2048:(half + 1) * 2048])
    wq8_sb = load_fp8_w(wq8, "wq8")
    wk8_sb = load_fp8_w(wk8, "wk8")
    xw8_sb = consts.tile([128, 2, T], FP8, name="xw8")
    xw8_v = xw8.rearrange("p (two t) -> p two t", two=2)
    for half in range(2):
        nc.sync.dma_start(
            out=xw8_sb[:, :, half * 2048:(half + 1) * 2048],
            in_=xw8_v[:, :, half * 2048:(half + 1) * 2048])

    xw_sb = consts.tile([128, 2, T], BF16, name="xw")
    nc.sync.dma_start(out=xw_sb, in_=xw.rearrange("(kc p) t -> p kc t", kc=2))
    wv_sb = consts.tile([128, 2, INNER], BF16, name="wv")
    nc.sync.dma_start(out=wv_sb, in_=wv.rearrange("(kc p) d -> p kc d", kc=2))
    xr_sb = consts.tile([128, 2, T], BF16, name="xr")
    nc.sync.dma_start(out=xr_sb, in_=xr.rearrange("(kc p) t -> p kc t", kc=2))
    hv_sb = consts.tile([128, 2, INNER], BF16, name="hv")
    nc.sync.dma_start(out=hv_sb, in_=hv.rearrange("(kc p) d -> p kc d", kc=2))
    wo_sb = consts.tile([128, 4, D], BF16, name="wo")
    nc.sync.dma_start(out=wo_sb, in_=wo.rearrange("(f p) d -> p f d", f=4))
    ho_sb = consts.tile([128, 4, D], BF16, name="ho")
    nc.sync.dma_start(out=ho_sb, in_=ho.rearrange("(f p) d -> p f d", f=4))

    ones_sb = consts.tile([128, 128], BF16, name="ones")
    nc.vector.memset(ones_sb, 1.0)

    # cross-phase tiles: partial tied logits out, AllReduce result in,
    # exp(logits) (computed during late phase B)
    dots_sb = dotsAp.tile([128, NH * 512], FP8, name="dots_sb")
    dotsr = dotsAp.tile([128, NH * 512], FP8, name="dotsr")
    Eh = dotsAp.tile([128, NH * 512], BF16, name="Eh")

    # ---------------------------------------------------------------
    # Phase A: height q/k projections (fp8 DoubleRow over the model dim) +
    # partial tied logits (fp8 DoubleRow over row-pairs); AllReduce.
    # dots^T[H](j, i) = sum_r sum_d k[r,j,H,d] q[r,i,H,d]  (j,i = columns)
    # ---------------------------------------------------------------
    with tc.tile_pool(name="phaseA", bufs=1) as phaseA, \
         tc.tile_pool(name="psA", bufs=2, space="PSUM") as psA, \
         tc.tile_pool(name="psDA", bufs=2, space="PSUM") as psDA:

        qhT, khT = [], []

        def emit_proj(f):
            for w8, outs, nm in ((hq8_sb, qhT, "qhT"), (hk8_sb, khT, "khT")):
                t = phaseA.tile([128, T], FP8, name=f"{nm}{f}")
                outs.append(t)
                for np2 in range(4):
                    ps = psA.tile([128, 1024], F32, tag="projA", name="projA")
                    for sub in range(2):
                        nt = np2 * 2 + sub
                        nc.tensor.matmul(
                            out=ps[:, sub * 512:(sub + 1) * 512],
                            lhsT=w8[:, :, f * 128:(f + 1) * 128],
                            rhs=xr8_sb[:, :, nt * 512:(nt + 1) * 512],
                            start=True, stop=True, perf_mode=DR)
                    copy_ps(t[:, np2 * 1024:(np2 + 1) * 1024], ps)

        def emit_hdots(f):
            # tied logits for heads 2f (bank 0) / 2f+1 (bank 1):
            # fp8 DoubleRow, contraction (row-pair, d) via [64, 2, n] APs
            dps = psDA.tile([128, 1024], F32, tag="hdots", name="hdots")
            kv = khT[f].rearrange("p (rp two jc j) -> p rp two jc j",
                                  rp=8, two=2, jc=2, j=128)
            qv = qhT[f].rearrange("p (rp two i) -> p rp two i",
                                  rp=8, two=2, i=256)
            for hp in range(2):
                b = hp * 64
                for jc in range(2):
                    for rp in range(8):
                        nc.tensor.matmul(
                            out=dps[:, hp * 512 + jc * 256:
                                    hp * 512 + (jc + 1) * 256],
                            lhsT=kv[b:b + 64, rp, :, jc, :],
                            rhs=qv[b:b + 64, rp, :, :],
                            start=(rp == 0), stop=(rp == 7),
                            perf_mode=DR)
            copy_ps(dots_sb[:, 2 * f * 512:(2 * f + 2) * 512], dps)

        # hdots(f) is emitted after proj(f+1) so the PE queue never waits on
        # the cast copies feeding it
        emit_proj(0)
        emit_proj(1)
        emit_hdots(0)
        emit_proj(2)
        emit_hdots(1)
        emit_proj(3)
        emit_hdots(2)
        emit_hdots(3)

        nc.sync.dma_start(out=cc_in[:, :], in_=dots_sb[:, :])
        nc.gpsimd.collective_compute(
            "AllReduce", mybir.AluOpType.add,
            replica_groups=[list(range(N_CORES))],
            ins=[cc_in.opt()], outs=[cc_out.opt()])
        # fetch the reduced logits on the gpsimd DGE queue (ordered after the
        # collective there; does not block the SP DMA queue)
        nc.gpsimd.dma_start(out=dotsr[:, :], in_=cc_out[:, :])

    # ---------------------------------------------------------------
    # Phase B: width attention over this core's 32 columns, with the height
    # v-projections, the width output projection, and the height exp
    # interleaved.  Ew slot layout: slot(H) = (H%2)*512 + (H//2)*128
    # ---------------------------------------------------------------
    NCG = 8                      # columns per group
    NGRP = WPC // NCG            # 4 groups
    GT = NCG * 128               # tokens per group (1024)

    vh = []                      # v (token-major) for the row shard
    w_out_v = w_out_t.rearrange("(mc p) t -> p mc t", mc=2)
    h_out_v = h_out_t.rearrange("(mc p) t -> p mc t", mc=2)

    with tc.tile_pool(name="phaseB", bufs=1) as phaseB, \
         tc.tile_pool(name="grpB", bufs=2) as grpB, \
         tc.tile_pool(name="colB", bufs=6) as colB, \
         tc.tile_pool(name="stgB", bufs=3) as stgB, \
         tc.tile_pool(name="psB", bufs=3, space="PSUM") as psB, \
         tc.tile_pool(name="psP", bufs=2, space="PSUM") as psP:

        # o^T accumulator: (128, f, tok) -- chunk f holds heads 2f, 2f+1
        owT = phaseB.tile([128, 4, T], BF16, name="owT")

        def emit_group_proj_chunks(g):
            """Thunks, each emitting one projection chunk for group g."""
            tok0 = g * GT
            qwT, kwT, vw = [], [], []
            thunks = []
            for f in range(4):
                for which, lst in ((0, qwT), (1, kwT)):
                    w8 = (wq8_sb, wk8_sb)[which]
                    t = grpB.tile([128, GT], FP8, tag=f"qk{which}{f}",
                                  name=f"qk{which}{f}")
                    lst.append(t)
                    for nt in range(GT // 512):
                        def th(w8=w8, t=t, nt=nt, f=f, tok0=tok0):
                            ps = psP.tile([128, 512], F32, tag="so512",
                                          name="projB")
                            nc.tensor.matmul(
                                out=ps,
                                lhsT=w8[:, :, f * 128:(f + 1) * 128],
                                rhs=xw8_sb[:, :, tok0 + nt * 512:
                                           tok0 + (nt + 1) * 512],
                                start=True, stop=True, perf_mode=DR)
                            copy_ps(t[:, nt * 512:(nt + 1) * 512], ps)
                        thunks.append(th)
            for ci in range(NCG):
                t = grpB.tile([128, INNER], BF16, tag=f"vw{ci}",
                              name=f"vw{ci}")
                vw.append(t)
                def th(t=t, ci=ci, tok0=tok0):
                    ps = psP.tile([128, 512], F32, tag="so512", name="projB")
                    for kc in range(2):
                        nc.tensor.matmul(
                            out=ps,
                            lhsT=xw_sb[:, kc, tok0 + ci * 128:
                                       tok0 + (ci + 1) * 128],
                            rhs=wv_sb[:, kc, :],
                            start=(kc == 0), stop=(kc == 1))
                    copy_ps(t, ps)
                thunks.append(th)
            return (qwT, kwT, vw), thunks

        def emit_vh_chunk(rc):
            t = vhp.tile([128, INNER], BF16, name=f"vh{rc}")
            vh.append(t)
            ps = psP.tile([128, 512], F32, tag="so512", name="projVH")
            for kc in range(2):
                nc.tensor.matmul(
                    out=ps,
                    lhsT=xr_sb[:, kc, rc * 128:(rc + 1) * 128],
                    rhs=hv_sb[:, kc, :],
                    start=(kc == 0), stop=(kc == 1))
            copy_ps(t, ps)

        def emit_wout_chunk(nt):
            # width output projection: w_out^T = wo^T @ o^T (tokens nt*512..)
            st = stgB.tile([128, 2, 512], BF16, tag="stgW", name="stgW")
            for mc in range(2):
                ps = psP.tile([128, 512], F32, tag="so512", name="oprojW")
                for f in range(4):
                    nc.tensor.matmul(
                        out=ps,
                        lhsT=wo_sb[:, f, mc * 128:(mc + 1) * 128],
                        rhs=owT[:, f, nt * 512:(nt + 1) * 512],
                        start=(f == 0), stop=(f == 3))
                copy_ps(st[:, mc, :], ps)
            nc.sync.dma_start(
                out=w_out_v[:, :, nt * 512:(nt + 1) * 512], in_=st)

        def emit_col(qwT, kwT, vw, g, ci):
            tok0 = g * GT
            c0 = ci * 128  # token offset within group
            # one PSUM tile per column: scores^T / exp / AV+denominators.
            # free = hp*512 + f*128 + i  (bank0 = hp0, bank1 = hp1)
            cps = psB.tile([128, 1024], F32, tag="colps", name="colps")
            for f in range(4):
                for hp in range(2):
                    b = hp * 64
                    nc.tensor.matmul(
                        out=cps[:, hp * 512 + f * 128:
                                hp * 512 + (f + 1) * 128],
                        lhsT=kwT[f][b:b + 64, c0:c0 + 128],
                        rhs=qwT[f][b:b + 64, c0:c0 + 128],
                        start=True, stop=True)
            Ew = colB.tile([128, 1024], BF16, tag="Ew", name="Ew")
            nc.scalar.activation(out=Ew, in_=cps, func=EXP, scale=SCALE)
            Binv2 = colB.tile([128, 512], F32, tag="Binv2", name="Binv2")
            # attn*V on UNNORMALIZED E overwrites bank 0 (f*128 blocks);
            # ones-matmul denominators overwrite bank 1 (hp partition-halves
            # replicated over 64 partitions so the normalize-mul broadcasts
            # for free). Both wait only on the exp.
            for hp in range(2):
                hb = hp * 512
                for f in range(4):
                    H = 2 * f + hp
                    nc.tensor.matmul(
                        out=cps[hp * 64:hp * 64 + 64,
                                f * 128:(f + 1) * 128],
                        lhsT=vw[ci][:, H * 64:(H + 1) * 64],
                        rhs=Ew[:, hb + f * 128:hb + (f + 1) * 128],
                        start=True, stop=True)
                nc.tensor.matmul(out=cps[hp * 64:hp * 64 + 64, 512:1024],
                                 lhsT=ones_sb[:, 0:64],
                                 rhs=Ew[:, hb:hb + 512],
                                 start=True, stop=True)
            nc.vector.reciprocal_approx_fast(out=Binv2, in_=cps[:, 512:1024])
            # o^T = o_unnorm * 1/s, fused with the PSUM->SBUF o^T copy
            nc.vector.tensor_mul(
                out=owT[:, :, tok0 + c0:tok0 + c0 + 128],
                in0=cps[:, 0:512].rearrange("p (f i) -> p f i", f=4),
                in1=Binv2.rearrange("p (f i) -> p f i", f=4))

        # software pipeline: group g's columns interleave with group g+1's
        # projections, the height v-projections, the width output projection,
        # and (in the last group) the height exp, so the PE and ACT/DVE
        # always have independent work queued
        cur_tiles, thunks = emit_group_proj_chunks(0)
        for th in thunks:
            th()
        for g in range(NGRP):
            nxt = None
            if g + 1 < NGRP:
                nxt_tiles, nxt_thunks = emit_group_proj_chunks(g + 1)
                nxt = iter(nxt_thunks)
                per_col = (len(nxt_thunks) + NCG - 1) // NCG
            for ci in range(NCG):
                gc = g * NCG + ci
                emit_col(*cur_tiles, g, ci)
                if nxt is not None:
                    for _ in range(per_col):
                        th = next(nxt, None)
                        if th is not None:
                            th()
                emit_vh_chunk(gc)
                if (gc + 1) % 4 == 0:
                    emit_wout_chunk((gc + 1) // 4 - 1)
            if nxt is not None:
                for th in nxt:
                    th()
                cur_tiles = nxt_tiles

        # height softmax numerators -- emitted after ALL latency-critical
        # ACT work so a late AllReduce cannot stall the in-order ACT queue
        for q8 in range(4):
            nc.scalar.activation(
                out=Eh[:, q8 * 1024:(q8 + 1) * 1024],
                in_=dotsr[:, q8 * 1024:(q8 + 1) * 1024],
                func=EXP, scale=TIE_SCALE)

    # ---------------------------------------------------------------
    # Phase C: height attention finish.
    # ---------------------------------------------------------------
    with tc.tile_pool(name="phaseC", bufs=1) as phaseC, \
         tc.tile_pool(name="stgC", bufs=3) as stgC, \
         tc.tile_pool(name="psOC", bufs=2, space="PSUM") as psOC, \
         tc.tile_pool(name="psSC", bufs=2, space="PSUM") as psSC, \
         tc.tile_pool(name="psHC", bufs=2, space="PSUM") as psHC:

        # denominators: B_H(i) = sum over both j-chunks and partitions.
        # BinvH2[p, f*256+i] = 1/s_{2f + (p>=64)}(i): parity-split partition
        # halves so the deferred normalize-mul below runs full-width.
        BinvH2 = phaseC.tile([128, 4 * 256], F32, name="BinvH2")
        for f in range(4):
            bps = psSC.tile([128, 256], F32, tag="bsumH", name="bsumH")
            for hp in range(2):
                H = 2 * f + hp
                p0 = hp * 64
                # complete each hp chain before the next starts (same bank)
                for jc in range(2):
                    nc.tensor.matmul(
                        out=bps[p0:p0 + 64, :],
                        lhsT=ones_sb[:, 0:64],
                        rhs=Eh[:, H * 512 + jc * 256:
                               H * 512 + (jc + 1) * 256],
                        start=(jc == 0), stop=(jc == 1))
            nc.vector.reciprocal_approx_fast(
                out=BinvH2[:, f * 256:(f + 1) * 256], in_=bps)

        # attn * V per row -> o^T chunks; ohT free = (f, r*256 + i)
        ohT = phaseC.tile([128, 4, T], BF16, name="ohT")
        for r in range(RPC):
            ops = psOC.tile([128, 1024], F32, tag="opsH", name="opsH")
            for f in range(4):
                for hp in range(2):
                    H = 2 * f + hp
                    for jc in range(2):
                        nc.tensor.matmul(
                            out=ops[hp * 64:hp * 64 + 64,
                                    f * 256:(f + 1) * 256],
                            lhsT=vh[r * 2 + jc][:, H * 64:(H + 1) * 64],
                            rhs=Eh[:, H * 512 + jc * 256:
                                   H * 512 + (jc + 1) * 256],
                            start=(jc == 0), stop=(jc == 1))
            nc.vector.tensor_mul(
                out=ohT[:, :, r * 256:(r + 1) * 256],
                in0=ops.rearrange("p (f i) -> p f i", f=4),
                in1=BinvH2.rearrange("p (f i) -> p f i", f=4))

            if r % 2 == 1:
                # height output projection for the two finished rows
                nt = r // 2
                st = stgC.tile([128, 2, 512], BF16, tag="stgH", name="stgH")
                for mc in range(2):
                    ps = psHC.tile([128, 512], F32, tag="oprojH",
                                   name="oprojH")
                    for f in range(4):
                        nc.tensor.matmul(
                            out=ps,
                            lhsT=ho_sb[:, f, mc * 128:(mc + 1) * 128],
                            rhs=ohT[:, f, nt * 512:(nt + 1) * 512],
                            start=(f == 0), stop=(f == 3))
                    copy_ps(st[:, mc, :], ps)
                nc.sync.dma_start(
                    out=h_out_v[:, :, nt * 512:(nt + 1) * 512], in_=st)

    ctx.close()


_NC = None


def _get_nc():
    global _NC
    if _NC is None:
        _NC = build_bass()
    return _NC


def _pair_interleave(a):
    """[256, n] fp32 -> [128, 2*n] fp8 pair-interleaved for DoubleRow."""
    n = a.shape[1]
    return np.ascontiguousarray(
        a.reshape(2, 128, n).transpose(1, 0, 2).reshape(128, 2 * n)
        .astype(NPFP8))


def make_in_maps(x, wq_w, wkv_w, wout_w, hq_w, hkv_w, hout_w):
    x4 = np.asarray(x, np.float32).reshape(H_ROWS, W_COLS, D)
    wq_w = np.asarray(wq_w, np.float32)
    wkv_w = np.asarray(wkv_w, np.float32)
    wout_w = np.asarray(wout_w, np.float32)
    hq_w = np.asarray(hq_w, np.float32)
    hkv_w = np.asarray(hkv_w, np.float32)
    hout_w = np.asarray(hout_w, np.float32)
    wghts = {
        "wq8": _pair_interleave(wq_w),
        "wk8": _pair_interleave(wkv_w[:, :INNER]),
        "hq8": _pair_interleave(hq_w),
        "hk8": _pair_interleave(hkv_w[:, :INNER]),
        "wv": np.ascontiguousarray(wkv_w[:, INNER:].astype(NPBF16)),
        "wo": np.ascontiguousarray(wout_w.astype(NPBF16)),
        "hv": np.ascontiguousarray(hkv_w[:, INNER:].astype(NPBF16)),
        "ho": np.ascontiguousarray(hout_w.astype(NPBF16)),
    }
    in_maps = []
    for c in range(N_CORES):
        xw_f = np.ascontiguousarray(
            x4[:, c * WPC:(c + 1) * WPC, :].transpose(1, 0, 2)
            .reshape(T, D).T)
        xr_f = np.ascontiguousarray(x4[c * RPC:(c + 1) * RPC].reshape(T, D).T)
        m = {"xw": xw_f.astype(NPBF16), "xr": xr_f.astype(NPBF16),
             "xw8": _pair_interleave(xw_f), "xr8": _pair_interleave(xr_f)}
        m.update(wghts)
        in_maps.append(m)
    return in_maps


def assemble_output(results, wout_b, hout_b):
    w_full = np.empty((H_ROWS, W_COLS, D), np.float32)
    h_full = np.empty((H_ROWS, W_COLS, D), np.float32)
    for c in range(N_CORES):
        wt = results[c]["w_out_t"].astype(np.float32)  # (256, 4096)
        w_full[:, c * WPC:(c + 1) * WPC, :] = \
            wt.T.reshape(WPC, H_ROWS, D).transpose(1, 0, 2)
        ht = results[c]["h_out_t"].astype(np.float32)
        h_full[c * RPC:(c + 1) * RPC] = ht.T.reshape(RPC, W_COLS, D)
    out = w_full + h_full
    out += (np.asarray(wout_b, np.float32) + np.asarray(hout_b, np.float32))
    return out.reshape(1, H_ROWS * W_COLS, D)


def kernel(x, wq_w, wkv_w, wout_w, wout_b, hq_w, hkv_w, hout_w, hout_b,
           msa_h=H_ROWS, msa_w=W_COLS, **_unused):
    in_maps = make_in_maps(x, wq_w, wkv_w, wout_w, hq_w, hkv_w, hout_w)
    nc = _get_nc()
    res = run_bass_kernel_spmd(nc, in_maps, core_ids=list(range(N_CORES)))
    return assemble_output(res.results, wout_b, hout_b)


# revision 18
# speedup vs baseline: 24.2142x; 24.2142x over previous
"""AlphaFold2 axial (row/column) MSA attention on 8 Trainium2 NeuronCores.

Problem: x (1, 32768, 256) = 128 MSA rows x 256 columns x dim 256.
  - width attention: softmax attention across the 128 rows, independent per
    column (256 independent length-128 sequences), 8 heads x 64.
  - height attention: "tied" attention across the 256 columns: logits are
    summed over all 128 rows, one (256x256) softmax per head shared by all
    rows.

Sharding (8 cores):
  - width: each core owns 32 columns (fully local).
  - height: each core owns 16 rows; per-core partial logits (8,256,256) are
    AllReduce'd (fp8, 0.5MB) across cores, softmax replicated, attn*V local.

Implementation (v4):
  - activations feature-major ("xT"), prepared host-side; projections and
    q.k^T need no on-device transpose.
  - ALL q/k projections run in fp8e4m3 with DoubleRow matmuls: the host
    supplies x and the q/k weights in the [128, 2, n] pair-interleaved
    layout, so one instruction contracts the full model dim (2x128) at 2x PE
    throughput.  q/k only feed softmax logits (~N(0, 0.1) after scaling), so
    fp8 error washes out in the near-uniform softmax; v and the output
    projections stay bf16 for accuracy.
  - the tied height logits additionally use DoubleRow to pack TWO MSA rows
    per instruction (contraction (row-pair, head-dim) via 3D APs on the fp8
    q^T/k^T tiles), and the AllReduce payload is fp8 (0.5MB).  hdots(f) is
    emitted behind the projections of f+1 so the in-order PE queue never
    stalls on the PSUM->SBUF cast copies; the AllReduce launches ~25us in.
  - width attention per column uses ONE [128, 1024] PSUM tile: dots -> one
    fused-scale exp -> attn*V overwrites bank 0 / ones-matmul denominators
    bank 1 -> reciprocal + normalize-multiply evacuate.  Normalization is
    deferred to the PSUM->SBUF copy; no max-subtraction.
  - the height v-projections, the width output projection, and the height
    exp(logits) are interleaved into the width column loop; the AllReduce
    result is fetched on the gpsimd DGE queue right after the collective.
  - DMAs are consolidated with 3D access patterns (one DMA per weight
    tensor, two per x variant, one per output chunk pair) because each
    DMACopy costs ~1us of in-order queue occupancy on top of the transfer.
  - outputs are written feature-major in bf16 and transposed/summed on host.

PSUM rules honored: a matmul accumulation chain fully finishes before
another chain's start=True touches the same bank (start clears has_written
bank-wide; data persists).
"""

import sys

for _p in ("/opt/trn_rl_repo",):
    if _p not in sys.path:
        sys.path.append(_p)

import numpy as np
import ml_dtypes

import concourse.bass as bass
import concourse.mybir as mybir
import concourse.tile as tile
from concourse import bacc
from concourse.bass_utils import run_bass_kernel_spmd

BF16 = mybir.dt.bfloat16
F32 = mybir.dt.float32
FP8 = mybir.dt.float8e4
NPBF16 = ml_dtypes.bfloat16
NPFP8 = ml_dtypes.float8_e4m3fn
EXP = mybir.ActivationFunctionType.Exp
DR = mybir.MatmulPerfMode.DoubleRow

N_CORES = 8
H_ROWS = 128          # MSA rows
W_COLS = 256          # sequence length (columns)
D = 256               # model dim
NH = 8                # heads
DH = 64               # head dim
INNER = NH * DH       # 512
WPC = W_COLS // N_CORES   # 32 columns per core
RPC = H_ROWS // N_CORES   # 16 rows per core
T = 4096              # tokens per shard (WPC*H_ROWS == RPC*W_COLS)
SCALE = DH ** -0.5                   # 0.125
TIE_SCALE = SCALE * (H_ROWS ** -0.5)


def _ap(h):
    return h.ap()


def build_bass(loop=1):
    nc = bacc.Bacc("TRN2", target_bir_lowering=False, debug=False,
                   num_devices=N_CORES)

    # ---- per-core I/O ----
    # bf16 activations (for the v projections) + fp8 pair-interleaved copies
    # (for the q/k projections); q/k weights only exist in fp8 form.
    xw = _ap(nc.dram_tensor("xw", [D, T], BF16, kind="ExternalInput"))
    xr = _ap(nc.dram_tensor("xr", [D, T], BF16, kind="ExternalInput"))
    xw8 = _ap(nc.dram_tensor("xw8", [128, 2 * T], FP8, kind="ExternalInput"))
    xr8 = _ap(nc.dram_tensor("xr8", [128, 2 * T], FP8, kind="ExternalInput"))
    wq8 = _ap(nc.dram_tensor("wq8", [128, 2 * INNER], FP8, kind="ExternalInput"))
    wk8 = _ap(nc.dram_tensor("wk8", [128, 2 * INNER], FP8, kind="ExternalInput"))
    hq8 = _ap(nc.dram_tensor("hq8", [128, 2 * INNER], FP8, kind="ExternalInput"))
    hk8 = _ap(nc.dram_tensor("hk8", [128, 2 * INNER], FP8, kind="ExternalInput"))
    wv = _ap(nc.dram_tensor("wv", [D, INNER], BF16, kind="ExternalInput"))
    wo = _ap(nc.dram_tensor("wo", [INNER, D], BF16, kind="ExternalInput"))
    hv = _ap(nc.dram_tensor("hv", [D, INNER], BF16, kind="ExternalInput"))
    ho = _ap(nc.dram_tensor("ho", [INNER, D], BF16, kind="ExternalInput"))
    w_out_t = _ap(nc.dram_tensor("w_out_t", [D, T], BF16, kind="ExternalOutput"))
    h_out_t = _ap(nc.dram_tensor("h_out_t", [D, T], BF16, kind="ExternalOutput"))

    with tile.TileContext(nc) as tc:
        for it in range(loop):
            # collective buffers must be distinct per unrolled iteration
            cc_in = _ap(nc.dram_tensor(f"cc_in{it}", [128, NH * 512], FP8,
                                       kind="Internal"))
            cc_out = _ap(nc.dram_tensor(f"cc_out{it}", [128, NH * 512], FP8,
                                        kind="Internal", addr_space="Shared"))
            build_tile_kernel(tc, xw, xr, xw8, xr8, wq8, wk8, hq8, hk8,
                              wv, wo, hv, ho, w_out_t, h_out_t, cc_in, cc_out)

    nc.compile()
    return nc


def build_tile_kernel(tc, xw, xr, xw8, xr8, wq8, wk8, hq8, hk8,
                      wv, wo, hv, ho, w_out_t, h_out_t, cc_in, cc_out):
    from contextlib import ExitStack

    nc = tc.nc
    ctx = ExitStack()

    # round-robin PSUM->SBUF copy engine (only ACT/DVE can read PSUM)
    _eng = [0]

    def copy_ps(out, in_):
        _eng[0] = (_eng[0] + 1) % 5
        if _eng[0] in (1, 3):
            nc.vector.tensor_copy(out=out, in_=in_)
        else:
            nc.scalar.copy(out=out, in_=in_)

    consts = ctx.enter_context(tc.tile_pool(name="consts", bufs=1))
    dotsAp = ctx.enter_context(tc.tile_pool(name="dotsAp", bufs=1))
    vhp = ctx.enter_context(tc.tile_pool(name="vhp", bufs=1))

    # ---- constants / weights into SBUF (phase-A dependencies first, few
    # large DMAs -- each DMACopy costs ~1us of in-order queue occupancy) ----
    def load_fp8_w(ap_in, name):  # [128, 2, INNER] pair-interleaved
        t = consts.tile([128, 2, INNER], FP8, name=name)
        nc.sync.dma_start(out=t, in_=ap_in)
        return t

    hq8_sb = load_fp8_w(hq8, "hq8")
    hk8_sb = load_fp8_w(hk8, "hk8")
    xr8_sb = consts.tile([128, 2, T], FP8, name="xr8")
    xr8_v = xr8.rearrange("p (two t) -> p two t", two=2)
    for half in range(2):
        nc.sync.dma_start(
            out=xr8_sb[:, :, half * 2048:(half + 1) * 2048],
            in_=xr8_v[:, :, half * 2048:(half + 1) * 2048])
    wq8_sb = load_fp8_w(wq8, "wq8")
    wk8_sb = load_fp8_w(wk8, "wk8")
    xw8_sb = consts.tile([128, 2, T], FP8, name="xw8")
    xw8_v = xw8.rearrange("p (two t) -> p two t", two=2)
    for half in range(2):
        nc.sync.dma_start(
            out=xw8_sb[:, :, half * 2048:(half + 1) * 2048],
            in_=xw8_v[:, :, half * 2048:(half + 1) * 2048])

    xw_sb = consts.tile([128, 2, T], BF16, name="xw")
    nc.sync.dma_start(out=xw_sb, in_=xw.rearrange("(kc p) t -> p kc t", kc=2))
    wv_sb = consts.tile([128, 2, INNER], BF16, name="wv")
    nc.sync.dma_start(out=wv_sb, in_=wv.rearrange("(kc p) d -> p kc d", kc=2))
    xr_sb = consts.tile([128, 2, T], BF16, name="xr")
    nc.sync.dma_start(out=xr_sb, in_=xr.rearrange("(kc p) t -> p kc t", kc=2))
    hv_sb = consts.tile([128, 2, INNER], BF16, name="hv")
    nc.sync.dma_start(out=hv_sb, in_=hv.rearrange("(kc p) d -> p kc d", kc=2))
    wo_sb = consts.tile([128, 4, D], BF16, name="wo")
    nc.sync.dma_start(out=wo_sb, in_=wo.rearrange("(f p) d -> p f d", f=4))
    ho_sb = consts.tile([128, 4, D], BF16, name="ho")
    nc.sync.dma_start(out=ho_sb, in_=ho.rearrange("(f p) d -> p f d", f=4))

    ones_sb = consts.tile([128, 128], BF16, name="ones")
    nc.vector.memset(ones_sb, 1.0)

    # cross-phase softmax denominators for the height attention (computed in
    # late phase B so phase C starts straight into attn*V)
    BinvH2 = dotsAp.tile([128, 4 * 256], F32, name="BinvH2")

    # cross-phase tiles: partial tied logits out, AllReduce result in,
    # exp(logits) (computed during late phase B)
    dots_sb = dotsAp.tile([128, NH * 512], FP8, name="dots_sb")
    dotsr = dotsAp.tile([128, NH * 512], FP8, name="dotsr")
    Eh = dotsAp.tile([128, NH * 512], BF16, name="Eh")

    # ---------------------------------------------------------------
    # Phase A: height q/k projections (fp8 DoubleRow over the model dim) +
    # partial tied logits (fp8 DoubleRow over row-pairs); AllReduce.
    # dots^T[H](j, i) = sum_r sum_d k[r,j,H,d] q[r,i,H,d]  (j,i = columns)
    # ---------------------------------------------------------------
    with tc.tile_pool(name="phaseA", bufs=1) as phaseA, \
         tc.tile_pool(name="psA", bufs=3, space="PSUM") as psA, \
         tc.tile_pool(name="psDA", bufs=1, space="PSUM") as psDA:

        # warm the PE (and its HAM clock-gate) with throwaway matmuls while
        # the first input DMAs land
        wps = psDA.tile([128, 1024], F32, tag="hdots", name="warm")
        for _ in range(12):
            nc.tensor.matmul(out=wps[:, 0:128], lhsT=ones_sb, rhs=ones_sb,
                             start=True, stop=True)

        qhT, khT = [], []

        def emit_proj(f):
            for w8, outs, nm in ((hq8_sb, qhT, "qhT"), (hk8_sb, khT, "khT")):
                t = phaseA.tile([128, T], FP8, name=f"{nm}{f}")
                outs.append(t)
                for np2 in range(4):
                    ps = psA.tile([128, 1024], F32, tag="projA", name="projA")
                    for sub in range(2):
                        nt = np2 * 2 + sub
                        nc.tensor.matmul(
                            out=ps[:, sub * 512:(sub + 1) * 512],
                            lhsT=w8[:, :, f * 128:(f + 1) * 128],
                            rhs=xr8_sb[:, :, nt * 512:(nt + 1) * 512],
                            start=True, stop=True, perf_mode=DR)
                    copy_ps(t[:, np2 * 1024:(np2 + 1) * 1024], ps)

        def emit_hdots(f):
            # tied logits for heads 2f (bank 0) / 2f+1 (bank 1):
            # fp8 DoubleRow, contraction (row-pair, d) via [64, 2, n] APs
            dps = psDA.tile([128, 1024], F32, tag="hdots", name="hdots")
            kv = khT[f].rearrange("p (rp two jc j) -> p rp two jc j",
                                  rp=8, two=2, jc=2, j=128)
            qv = qhT[f].rearrange("p (rp two i) -> p rp two i",
                                  rp=8, two=2, i=256)
            for hp in range(2):
                b = hp * 64
                for jc in range(2):
                    for rp in range(8):
                        nc.tensor.matmul(
                            out=dps[:, hp * 512 + jc * 256:
                                    hp * 512 + (jc + 1) * 256],
                            lhsT=kv[b:b + 64, rp, :, jc, :],
                            rhs=qv[b:b + 64, rp, :, :],
                            start=(rp == 0), stop=(rp == 7),
                            perf_mode=DR)
            copy_ps(dots_sb[:, 2 * f * 512:(2 * f + 2) * 512], dps)

        # hdots(f) is emitted after proj(f+1) so the PE queue never waits on
        # the cast copies feeding it
        emit_proj(0)
        emit_proj(1)
        emit_hdots(0)
        emit_proj(2)
        emit_hdots(1)
        emit_proj(3)
        emit_hdots(2)
        emit_hdots(3)

        nc.sync.dma_start(out=cc_in[:, :], in_=dots_sb[:, :])
        nc.gpsimd.collective_compute(
            "AllReduce", mybir.AluOpType.add,
            replica_groups=[list(range(N_CORES))],
            ins=[cc_in.opt()], outs=[cc_out.opt()])
        # fetch the reduced logits on the gpsimd DGE queue (ordered after the
        # collective there; does not block the SP DMA queue)
        nc.gpsimd.dma_start(out=dotsr[:, :], in_=cc_out[:, :])

    # ---------------------------------------------------------------
    # Phase B: width attention over this core's 32 columns, with the height
    # v-projections, the width output projection, and the height exp
    # interleaved.  Ew slot layout: slot(H) = (H%2)*512 + (H//2)*128
    # ---------------------------------------------------------------
    NCG = 8                      # columns per group
    NGRP = WPC // NCG            # 4 groups
    GT = NCG * 128               # tokens per group (1024)

    vh = []                      # v (token-major) for the row shard
    w_out_v = w_out_t.rearrange("(mc p) t -> p mc t", mc=2)
    h_out_v = h_out_t.rearrange("(mc p) t -> p mc t", mc=2)

    with tc.tile_pool(name="phaseB", bufs=1) as phaseB, \
         tc.tile_pool(name="grpB", bufs=2) as grpB, \
         tc.tile_pool(name="colB", bufs=6) as colB, \
         tc.tile_pool(name="stgB", bufs=3) as stgB, \
         tc.tile_pool(name="psB", bufs=3, space="PSUM") as psB, \
         tc.tile_pool(name="psP", bufs=2, space="PSUM") as psP:

        # o^T accumulator: (128, f, tok) -- chunk f holds heads 2f, 2f+1
        owT = phaseB.tile([128, 4, T], BF16, name="owT")

        def emit_group_proj_chunks(g):
            """Thunks, each emitting one projection chunk for group g."""
            tok0 = g * GT
            qwT, kwT, vw = [], [], []
            thunks = []
            for f in range(4):
                for which, lst in ((0, qwT), (1, kwT)):
                    w8 = (wq8_sb, wk8_sb)[which]
                    t = grpB.tile([128, GT], FP8, tag=f"qk{which}{f}",
                                  name=f"qk{which}{f}")
                    lst.append(t)
                    for nt in range(GT // 512):
                        def th(w8=w8, t=t, nt=nt, f=f, tok0=tok0):
                            ps = psP.tile([128, 512], F32, tag="so512",
                                          name="projB")
                            nc.tensor.matmul(
                                out=ps,
                                lhsT=w8[:, :, f * 128:(f + 1) * 128],
                                rhs=xw8_sb[:, :, tok0 + nt * 512:
                                           tok0 + (nt + 1) * 512],
                                start=True, stop=True, perf_mode=DR)
                            copy_ps(t[:, nt * 512:(nt + 1) * 512], ps)
                        thunks.append(th)
            for ci in range(NCG):
                t = grpB.tile([128, INNER], BF16, tag=f"vw{ci}",
                              name=f"vw{ci}")
                vw.append(t)
                def th(t=t, ci=ci, tok0=tok0):
                    ps = psP.tile([128, 512], F32, tag="so512", name="projB")
                    for kc in range(2):
                        nc.tensor.matmul(
                            out=ps,
                            lhsT=xw_sb[:, kc, tok0 + ci * 128:
                                       tok0 + (ci + 1) * 128],
                            rhs=wv_sb[:, kc, :],
                            start=(kc == 0), stop=(kc == 1))
                    copy_ps(t, ps)
                thunks.append(th)
            return (qwT, kwT, vw), thunks

        def emit_vh_chunk(rc):
            t = vhp.tile([128, INNER], BF16, name=f"vh{rc}")
            vh.append(t)
            ps = psP.tile([128, 512], F32, tag="so512", name="projVH")
            for kc in range(2):
                nc.tensor.matmul(
                    out=ps,
                    lhsT=xr_sb[:, kc, rc * 128:(rc + 1) * 128],
                    rhs=hv_sb[:, kc, :],
                    start=(kc == 0), stop=(kc == 1))
            copy_ps(t, ps)

        def emit_wout_chunk(nt):
            # width output projection: w_out^T = wo^T @ o^T (tokens nt*512..)
            st = stgB.tile([128, 2, 512], BF16, tag="stgW", name="stgW")
            for mc in range(2):
                ps = psP.tile([128, 512], F32, tag="so512", name="oprojW")
                for f in range(4):
                    nc.tensor.matmul(
                        out=ps,
                        lhsT=wo_sb[:, f, mc * 128:(mc + 1) * 128],
                        rhs=owT[:, f, nt * 512:(nt + 1) * 512],
                        start=(f == 0), stop=(f == 3))
                copy_ps(st[:, mc, :], ps)
            nc.sync.dma_start(
                out=w_out_v[:, :, nt * 512:(nt + 1) * 512], in_=st)

        def emit_col(qwT, kwT, vw, g, ci):
            tok0 = g * GT
            c0 = ci * 128  # token offset within group
            # one PSUM tile per column: scores^T / exp / AV+denominators.
            # free = hp*512 + f*128 + i  (bank0 = hp0, bank1 = hp1)
            cps = psB.tile([128, 1024], F32, tag="colps", name="colps")
            for f in range(4):
                for hp in range(2):
                    b = hp * 64
                    nc.tensor.matmul(
                        out=cps[:, hp * 512 + f * 128:
                                hp * 512 + (f + 1) * 128],
                        lhsT=kwT[f][b:b + 64, c0:c0 + 128],
                        rhs=qwT[f][b:b + 64, c0:c0 + 128],
                        start=True, stop=True)
            Ew = colB.tile([128, 1024], BF16, tag="Ew", name="Ew")
            nc.scalar.activation(out=Ew, in_=cps, func=EXP, scale=SCALE)
            Binv2 = colB.tile([128, 512], F32, tag="Binv2", name="Binv2")
            # attn*V on UNNORMALIZED E overwrites bank 0 (f*128 blocks);
            # ones-matmul denominators overwrite bank 1 (hp partition-halves
            # replicated over 64 partitions so the normalize-mul broadcasts
            # for free). Both wait only on the exp.
            for hp in range(2):
                hb = hp * 512
                for f in range(4):
                    H = 2 * f + hp
                    nc.tensor.matmul(
                        out=cps[hp * 64:hp * 64 + 64,
                                f * 128:(f + 1) * 128],
                        lhsT=vw[ci][:, H * 64:(H + 1) * 64],
                        rhs=Ew[:, hb + f * 128:hb + (f + 1) * 128],
                        start=True, stop=True)
                nc.tensor.matmul(out=cps[hp * 64:hp * 64 + 64, 512:1024],
                                 lhsT=ones_sb[:, 0:64],
                                 rhs=Ew[:, hb:hb + 512],
                                 start=True, stop=True)
            nc.vector.reciprocal_approx_fast(out=Binv2, in_=cps[:, 512:1024])
            # o^T = o_unnorm * 1/s, fused with the PSUM->SBUF o^T copy
            nc.vector.tensor_mul(
                out=owT[:, :, tok0 + c0:tok0 + c0 + 128],
                in0=cps[:, 0:512].rearrange("p (f i) -> p f i", f=4),
                in1=Binv2.rearrange("p (f i) -> p f i", f=4))

        def emit_Eh(q8):
            # height softmax numerators; emitted deep enough into phase B
            # that the AllReduce result is certainly in SBUF (an in-order
            # ACT queue stall would starve the column pipeline)
            nc.scalar.activation(
                out=Eh[:, q8 * 1024:(q8 + 1) * 1024],
                in_=dotsr[:, q8 * 1024:(q8 + 1) * 1024],
                func=EXP, scale=TIE_SCALE)

        def emit_hbsum(fp):
            # height denominators: B_H(i) sums both j-chunks and partitions.
            # BinvH2[p, f*256+i] = 1/s_{2f + (p>=64)}(i): parity-split
            # partition halves so phase C's normalize-mul runs full-width.
            bps = psP.tile([128, 512], F32, tag="so512", name="bsumH")
            for sub in range(2):
                f = fp * 2 + sub
                for hp in range(2):
                    H = 2 * f + hp
                    p0 = hp * 64
                    # complete each chain before the next start in this bank
                    for jc in range(2):
                        nc.tensor.matmul(
                            out=bps[p0:p0 + 64, sub * 256:(sub + 1) * 256],
                            lhsT=ones_sb[:, 0:64],
                            rhs=Eh[:, H * 512 + jc * 256:
                                   H * 512 + (jc + 1) * 256],
                            start=(jc == 0), stop=(jc == 1))
            nc.vector.reciprocal_approx_fast(
                out=BinvH2[:, fp * 512:(fp + 1) * 512], in_=bps)

        # software pipeline: group g's columns interleave with group g+1's
        # projections, the height v-projections, the width output projection,
        # and (late) the height softmax numerators/denominators, so the PE
        # and ACT/DVE always have independent work queued
        cur_tiles, thunks = emit_group_proj_chunks(0)
        for th in thunks:
            th()
        for g in range(NGRP):
            nxt = None
            if g + 1 < NGRP:
                nxt_tiles, nxt_thunks = emit_group_proj_chunks(g + 1)
                nxt = iter(nxt_thunks)
                per_col = (len(nxt_thunks) + NCG - 1) // NCG
            for ci in range(NCG):
                gc = g * NCG + ci
                emit_col(*cur_tiles, g, ci)
                if nxt is not None:
                    for _ in range(per_col):
                        th = next(nxt, None)
                        if th is not None:
                            th()
                emit_vh_chunk(gc)
                # wout(nt) is emitted two columns after its last input column
                # so the PE never waits on that column's normalize-mul
                if gc >= 5 and (gc - 5) % 4 == 0:
                    emit_wout_chunk((gc - 5) // 4)
            if nxt is not None:
                for th in nxt:
                    th()
                cur_tiles = nxt_tiles
        emit_wout_chunk(NGRP * NCG // 4 - 1)
        for q8 in range(4):
            emit_Eh(q8)
        for fp in range(2):
            emit_hbsum(fp)

    # ---------------------------------------------------------------
    # Phase C: height attention finish.
    # ---------------------------------------------------------------
    with tc.tile_pool(name="phaseC", bufs=1) as phaseC, \
         tc.tile_pool(name="stgC", bufs=3) as stgC, \
         tc.tile_pool(name="psOC", bufs=3, space="PSUM") as psOC, \
         tc.tile_pool(name="psHC", bufs=1, space="PSUM") as psHC:

        def emit_hout(t0, n):
            # height output projection for finished tokens [t0, t0+n)
            ps = psHC.tile([128, 1024], F32, tag="oprojH", name="oprojH")
            for mc in range(2):
                for f in range(4):
                    nc.tensor.matmul(
                        out=ps[:, mc * n:mc * n + n],
                        lhsT=ho_sb[:, f, mc * 128:(mc + 1) * 128],
                        rhs=ohT[:, f, t0:t0 + n],
                        start=(f == 0), stop=(f == 3))
            st = stgC.tile([128, 2, n], BF16, tag=f"stgH{n}", name="stgH")
            copy_ps(st, ps[:, 0:2 * n].rearrange("p (mc i) -> p mc i", mc=2))
            nc.sync.dma_start(out=h_out_v[:, :, t0:t0 + n], in_=st)

        # attn * V per row -> o^T chunks; ohT free = (f, r*256 + i)
        ohT = phaseC.tile([128, 4, T], BF16, name="ohT")
        for r in range(RPC):
            ops = psOC.tile([128, 1024], F32, tag="opsH", name="opsH")
            for f in range(4):
                for hp in range(2):
                    H = 2 * f + hp
                    for jc in range(2):
                        nc.tensor.matmul(
                            out=ops[hp * 64:hp * 64 + 64,
                                    f * 256:(f + 1) * 256],
                            lhsT=vh[r * 2 + jc][:, H * 64:(H + 1) * 64],
                            rhs=Eh[:, H * 512 + jc * 256:
                                   H * 512 + (jc + 1) * 256],
                            start=(jc == 0), stop=(jc == 1))
            nc.vector.tensor_mul(
                out=ohT[:, :, r * 256:(r + 1) * 256],
                in0=ops.rearrange("p (f i) -> p f i", f=4),
                in1=BinvH2.rearrange("p (f i) -> p f i", f=4))

            if r >= RPC - 2:
                # final rows at single-row granularity for a shorter tail
                emit_hout(r * 256, 256)
            elif r % 2 == 1:
                emit_hout((r // 2) * 512, 512)

    ctx.close()


_NC = None


def _get_nc():
    global _NC
    if _NC is None:
        _NC = build_bass()
    return _NC


def _pair_interleave(a):
    """[256, n] fp32 -> [128, 2*n] fp8 pair-interleaved for DoubleRow."""
    n = a.shape[1]
    return np.ascontiguousarray(
        a.reshape(2, 128, n).transpose(1, 0, 2).reshape(128, 2 * n)
        .astype(NPFP8))


def make_in_maps(x, wq_w, wkv_w, wout_w, hq_w, hkv_w, hout_w):
    x4 = np.asarray(x, np.float32).reshape(H_ROWS, W_COLS, D)
    wq_w = np.asarray(wq_w, np.float32)
    wkv_w = np.asarray(wkv_w, np.float32)
    wout_w = np.asarray(wout_w, np.float32)
    hq_w = np.asarray(hq_w, np.float32)
    hkv_w = np.asarray(hkv_w, np.float32)
    hout_w = np.asarray(hout_w, np.float32)
    wghts = {
        "wq8": _pair_interleave(wq_w),
        "wk8": _pair_interleave(wkv_w[:, :INNER]),
        "hq8": _pair_interleave(hq_w),
        "hk8": _pair_interleave(hkv_w[:, :INNER]),
        "wv": np.ascontiguousarray(wkv_w[:, INNER:].astype(NPBF16)),
        "wo": np.ascontiguousarray(wout_w.astype(NPBF16)),
        "hv": np.ascontiguousarray(hkv_w[:, INNER:].astype(NPBF16)),
        "ho": np.ascontiguousarray(hout_w.astype(NPBF16)),
    }
    in_maps = []
    for c in range(N_CORES):
        xw_f = np.ascontiguousarray(
            x4[:, c * WPC:(c + 1) * WPC, :].transpose(1, 0, 2)
            .reshape(T, D).T)
        xr_f = np.ascontiguousarray(x4[c * RPC:(c + 1) * RPC].reshape(T, D).T)
        m = {"xw": xw_f.astype(NPBF16), "xr": xr_f.astype(NPBF16),
             "xw8": _pair_interleave(xw_f), "xr8": _pair_interleave(xr_f)}
        m.update(wghts)
        in_maps.append(m)
    return in_maps


def assemble_output(results, wout_b, hout_b):
    w_full = np.empty((H_ROWS, W_COLS, D), np.float32)
    h_full = np.empty((H_ROWS, W_COLS, D), np.float32)
    for c in range(N_CORES):
        wt = results[c]["w_out_t"].astype(np.float32)  # (256, 4096)
        w_full[:, c * WPC:(c + 1) * WPC, :] = \
            wt.T.reshape(WPC, H_ROWS, D).transpose(1, 0, 2)
        ht = results[c]["h_out_t"].astype(np.float32)
        h_full[c * RPC:(c + 1) * RPC] = ht.T.reshape(RPC, W_COLS, D)
    out = w_full + h_full
    out += (np.asarray(wout_b, np.float32) + np.asarray(hout_b, np.float32))
    return out.reshape(1, H_ROWS * W_COLS, D)


def kernel(x, wq_w, wkv_w, wout_w, wout_b, hq_w, hkv_w, hout_w, hout_b,
           msa_h=H_ROWS, msa_w=W_COLS, **_unused):
    in_maps = make_in_maps(x, wq_w, wkv_w, wout_w, hq_w, hkv_w, hout_w)
    nc = _get_nc()
    res = run_bass_kernel_spmd(nc, in_maps, core_ids=list(range(N_CORES)))
    return assemble_output(res.results, wout_b, hout_b)
